# revision 1
# baseline (speedup 1.0000x reference)
"""GAT message-passing kernel for 8 TRN2 NeuronCores (Bass/Tile).

Strategy (dst-sharded, no collectives):
  - Each core owns a contiguous range of destination nodes; the host routes
    each edge to the core owning its destination (edge_index[1]).
  - Device phase A: build a node table T[row n] = [x[n] bf16 x64 | 1.0 | s_j |
    s_i | 0] (68 bf16 = 136B rows) in HBM. s_j/s_i = x @ w_j / x @ w_i are
    computed on TensorE from a host-transposed copy of x.
  - The host groups each core's edges into per-destination blocks of
    S = deg+1 slots (slot 0 gathers the destination's own row, providing
    s_i), buckets nodes by exact degree, packs blocks into 128-slot tiles,
    and emits one int32 table-row index per slot.
  - Device phase B: [128,1] indirect-DMA gathers pull 128 slot rows each
    into SBUF; ScalarE computes exp(leakyrelu(s_i + s_j)); TensorE
    mask-matmuls reduce each tile to per-node numerators (sum ex*x) and
    denominators (sum ex); divide + relu; DMA out.
  - Host inverts the block permutation and assembles the full output.
"""
import numpy as np

N_NODES = 100000
HIDDEN = 64
N_CORES = 8
LEAKY = 0.01
P = 128
BCOLS = 784              # table columns per partition (783 data + 1 sentinel)
NPAD = P * (BCOLS - 1)   # 100224 padded node count
ROW = 68                 # bf16 elements per table row (136B)
SENTINEL = BCOLS - 1     # flat table row 783 (partition 0, col 783)
TCHUNK = 16              # table-build chunk (columns per iteration)
NB = 12                  # gather tiles per sub-batch (ring-capacity pacing)
NQUEUES = 1              # SWDGE queues to rotate gathers over
NEG_BIG = -1.0e30


def _phi(n):
    """node id -> flat table row index (partition-major storage)."""
    return (n % P) * BCOLS + n // P


def _build_layout(edge_src, edge_dst_local, nodes_per_core):
    ncores = len(edge_src)
    blocks = {}
    for c in range(ncores):
        src, dstl = edge_src[c], edge_dst_local[c]
        order = np.argsort(dstl, kind="stable")
        src, dstl = src[order], dstl[order]
        deg = np.bincount(dstl, minlength=nodes_per_core)
        starts = np.concatenate([[0], np.cumsum(deg)])
        per = {}
        for n in np.nonzero(deg)[0]:
            d = int(deg[n])
            per.setdefault(d, []).append((int(n), src[starts[n]:starts[n + 1]]))
        blocks[c] = per

    all_d = sorted({d for c in range(ncores) for d in blocks[c].keys()})
    bucket_list = []
    for d in all_d:
        if d <= 0 or d > 126:
            raise ValueError(f"unsupported degree {d}")
        S = d + 1
        m = P // S
        maxb = max(len(blocks[c].get(d, [])) for c in range(ncores))
        n_tiles = (maxb + m - 1) // m
        bucket_list.append((d, n_tiles, m))

    total_tiles = sum(t for _, t, _ in bucket_list)
    total_cols = sum(t * m for _, t, m in bucket_list)
    Js, colmaps = [], []
    for c in range(ncores):
        J = np.full((total_tiles, P), SENTINEL, dtype=np.int32)
        colmap = np.full(total_cols, -1, dtype=np.int32)
        k0 = 0
        c0 = 0
        base_global = c * nodes_per_core
        for d, n_tiles, m in bucket_list:
            S = d + 1
            for bi, (n, srcs) in enumerate(blocks[c].get(d, [])):
                t, b = bi // m, bi % m
                J[k0 + t, b * S] = _phi(base_global + n)
                J[k0 + t, b * S + 1: b * S + 1 + d] = _phi(srcs)
                colmap[c0 + t * m + b] = n
            k0 += n_tiles
            c0 += n_tiles * m
        Js.append(np.ascontiguousarray(J.T))
        colmaps.append(colmap)
    return bucket_list, total_tiles, total_cols, Js, colmaps


def _build_masks(bucket_list):
    import ml_dtypes

    bm, sm = [], []
    for d, _, m in bucket_list:
        S = d + 1
        B = np.zeros((P, m), dtype=np.float32)
        SEL = np.zeros((P, P), dtype=np.float32)
        for p in range(m * S):
            if p % S != 0:
                B[p, p // S] = 1.0
            SEL[(p // S) * S, p] = 1.0
        bm.append(B)
        sm.append(SEL)
    return (np.concatenate(bm, 1).astype(ml_dtypes.bfloat16),
            np.concatenate(sm, 1).astype(ml_dtypes.bfloat16))


def _build_program(bucket_list, total_tiles, total_cols, n_bm_cols):
    import concourse.bass as bass
    import concourse.tile as tile
    from concourse import bacc, mybir
    from concourse.mybir import ActivationFunctionType as AFT

    nc = bacc.Bacc("TRN2", target_bir_lowering=False,
                   num_swdge_queues=NQUEUES,
                   dynamic_dma_scratch_size=65536)
    XR = nc.dram_tensor("XR", [P, BCOLS * HIDDEN], mybir.dt.float32,
                        kind="ExternalInput")
    XT = nc.dram_tensor("XT", [HIDDEN, P * BCOLS], mybir.dt.float32,
                        kind="ExternalInput")
    W2 = nc.dram_tensor("W2", [HIDDEN, 2], mybir.dt.float32,
                        kind="ExternalInput")
    JT = nc.dram_tensor("JT", [P, total_tiles], mybir.dt.int32,
                        kind="ExternalInput")
    BM = nc.dram_tensor("BM", [P, n_bm_cols], mybir.dt.bfloat16,
                        kind="ExternalInput")
    SM = nc.dram_tensor("SM", [P, P * len(bucket_list)], mybir.dt.bfloat16,
                        kind="ExternalInput")
    T = nc.dram_tensor("T", [P, BCOLS * ROW], mybir.dt.bfloat16)
    OUT = nc.dram_tensor("OUT", [HIDDEN + 1, total_cols], mybir.dt.float32,
                         kind="ExternalOutput")

    Trows = T[:].rearrange("p (b c) -> (p b) c", c=ROW)
    Tview = T[:].rearrange("p (b c) -> p b c", c=ROW)

    with tile.TileContext(nc) as tc:
        # ---------------- phase A: build node table ----------------
        with (
            tc.tile_pool(name="xa", bufs=2) as xa,
            tc.tile_pool(name="xt", bufs=2) as xtp,
            tc.tile_pool(name="stg", bufs=2) as stg,
            tc.tile_pool(name="wp", bufs=1) as wp,
            tc.tile_pool(name="psA", bufs=2, space="PSUM") as psA,
        ):
            w2f = wp.tile([HIDDEN, 2], mybir.dt.float32)
            nc.sync.dma_start(w2f[:], W2[:])
            w2b = wp.tile([HIDDEN, 2], mybir.dt.bfloat16)
            nc.vector.tensor_copy(w2b[:], w2f[:])

            for it in range(BCOLS // TCHUNK):
                b0 = it * TCHUNK
                xin = xa.tile([P, TCHUNK * HIDDEN], mybir.dt.float32)
                nc.sync.dma_start(
                    xin[:], XR[:, b0 * HIDDEN:(b0 + TCHUNK) * HIDDEN])
                xtin = xtp.tile([HIDDEN, TCHUNK * P], mybir.dt.float32)
                nc.sync.dma_start(xtin[:], XT[:, b0 * P:(b0 + TCHUNK) * P])
                xtb = xtp.tile([HIDDEN, TCHUNK * P], mybir.dt.bfloat16)
                nc.vector.tensor_copy(xtb[:], xtin[:])
                ps = psA.tile([P, 2 * TCHUNK], mybir.dt.float32)
                for j in range(TCHUNK):
                    nc.tensor.matmul(
                        ps[:, 2 * j:2 * j + 2],
                        lhsT=xtb[:, j * P:(j + 1) * P],
                        rhs=w2b[:],
                        start=True, stop=True)
                st = stg.tile([P, TCHUNK, ROW], mybir.dt.bfloat16)
                nc.vector.memset(st[:, :, 0:1], 1.0)
                nc.vector.tensor_copy(
                    st[:, :, 1:HIDDEN + 1],
                    xin[:].rearrange("p (t h) -> p t h", h=HIDDEN))
                nc.vector.tensor_copy(
                    st[:, :, HIDDEN + 1:HIDDEN + 3],
                    ps[:].rearrange("p (t s) -> p t s", s=2))
                nc.vector.memset(st[:, :, HIDDEN + 3:ROW], 0.0)
                nc.sync.dma_start(
                    T[:, b0 * ROW:(b0 + TCHUNK) * ROW],
                    st[:].rearrange("p t c -> p (t c)"))

            sent = wp.tile([P, 1], mybir.dt.bfloat16)
            nc.vector.memset(sent[:], NEG_BIG)
            nc.sync.dma_start(
                Tview[:, SENTINEL, HIDDEN + 1:HIDDEN + 2], sent[:])

        # ---------------- phase B: gather + softmax + reduce --------
        with (
            tc.tile_pool(name="msk", bufs=1) as mskp,
            tc.tile_pool(name="jt", bufs=2) as jtp,
            tc.tile_pool(name="g", bufs=3) as gp,
            tc.tile_pool(name="sc", bufs=4) as scp,
            tc.tile_pool(name="fl", bufs=4) as flp,
            tc.tile_pool(name="psS", bufs=2, space="PSUM") as psS,
            tc.tile_pool(name="psU", bufs=2, space="PSUM") as psU,
        ):
            bmall = mskp.tile([P, n_bm_cols], mybir.dt.bfloat16)
            nc.sync.dma_start(bmall[:], BM[:])
            small = mskp.tile([P, P * len(bucket_list)], mybir.dt.bfloat16)
            nc.sync.dma_start(small[:], SM[:])
            ones1 = mskp.tile([1, HIDDEN + 1], mybir.dt.bfloat16)
            nc.vector.memset(ones1[:], 1.0)

            k0 = 0
            c0 = 0
            bm0 = 0
            for bi, (d, n_tiles, m) in enumerate(bucket_list):
                t = 0
                while t < n_tiles:
                    nb = min(NB, max(1, 512 // m), n_tiles - t)
                    jt = jtp.tile([P, NB], mybir.dt.int32, tag="jt")
                    nc.sync.dma_start(jt[:, :nb], JT[:, k0 + t:k0 + t + nb])
                    G = gp.tile([P, NB, ROW], mybir.dt.bfloat16, tag="G")
                    for k in range(nb):
                        gi = nc.gpsimd.indirect_dma_start(
                            out=G[:, k, :],
                            out_offset=None,
                            in_=Trows,
                            in_offset=bass.IndirectOffsetOnAxis(
                                ap=jt[:, k:k + 1], axis=0))
                        q = k % NQUEUES
                        if q:
                            gi.queue = f"qPoolDynamic{q}"
                    Vc = scp.tile([P, NB], mybir.dt.bfloat16, tag="Vc")
                    nc.vector.tensor_copy(Vc[:, :nb], G[:, :nb, HIDDEN + 2])
                    sib = psS.tile([P, NB], mybir.dt.float32, tag="sib")
                    nc.tensor.matmul(
                        sib[:, :nb],
                        lhsT=small[:, bi * P:(bi + 1) * P],
                        rhs=Vc[:, :nb],
                        start=True, stop=True)
                    eraw = scp.tile([P, NB], mybir.dt.float32, tag="eraw")
                    nc.vector.tensor_add(
                        eraw[:, :nb], sib[:, :nb], G[:, :nb, HIDDEN + 1])
                    esc = scp.tile([P, NB], mybir.dt.float32, tag="esc")
                    nc.vector.tensor_scalar_mul(esc[:, :nb], eraw[:, :nb],
                                                LEAKY)
                    elr = scp.tile([P, NB], mybir.dt.float32, tag="elr")
                    nc.vector.tensor_max(elr[:, :nb], eraw[:, :nb],
                                         esc[:, :nb])
                    ex = scp.tile([P, NB], mybir.dt.float32, tag="ex")
                    nc.scalar.activation(ex[:, :nb], elr[:, :nb], AFT.Exp)
                    exsel = scp.tile([P, NB, m], mybir.dt.bfloat16,
                                     tag="exsel")
                    nc.vector.tensor_mul(
                        exsel[:, :nb, :],
                        bmall[:, bm0:bm0 + m].unsqueeze(1).broadcast_to(
                            [P, nb, m]),
                        ex[:, :nb].unsqueeze(2).broadcast_to([P, nb, m]))
                    U = psU.tile([HIDDEN + 1, 512], mybir.dt.float32,
                                 tag="U")
                    for k in range(nb):
                        nc.tensor.matmul(
                            U[:, k * m:(k + 1) * m],
                            lhsT=G[:, k, 0:HIDDEN + 1],
                            rhs=exsel[:, k, :],
                            start=True, stop=True)
                    dsb = flp.tile([1, 512], mybir.dt.float32, tag="dsb")
                    nc.vector.tensor_scalar_max(
                        dsb[:, :nb * m], U[0:1, :nb * m], 1e-30)
                    recf = flp.tile([1, 512], mybir.dt.float32, tag="recf")
                    nc.vector.reciprocal(recf[:, :nb * m], dsb[:, :nb * m])
                    rec = flp.tile([1, 512], mybir.dt.bfloat16, tag="rec")
                    nc.vector.tensor_copy(rec[:, :nb * m], recf[:, :nb * m])
                    rb = psS.tile([HIDDEN + 1, 512], mybir.dt.float32,
                                  tag="rb")
                    nc.tensor.matmul(
                        rb[:, :nb * m], lhsT=ones1[:], rhs=rec[:, :nb * m],
                        start=True, stop=True)
                    rbs = flp.tile([HIDDEN + 1, 512], mybir.dt.float32,
                                   tag="rbs")
                    nc.scalar.copy(rbs[:, :nb * m], rb[:, :nb * m])
                    ot = flp.tile([HIDDEN + 1, 512], mybir.dt.float32,
                                  tag="ot")
                    nc.vector.tensor_mul(
                        ot[:, :nb * m], U[:, :nb * m], rbs[:, :nb * m])
                    otr = flp.tile([HIDDEN + 1, 512], mybir.dt.float32,
                                   tag="otr")
                    nc.vector.tensor_scalar_max(
                        otr[:, :nb * m], ot[:, :nb * m], 0.0)
                    nc.sync.dma_start(
                        OUT[:, c0:c0 + nb * m], otr[:, :nb * m])
                    c0 += nb * m
                    t += nb
                k0 += n_tiles
                bm0 += m
    nc.compile()
    return nc


def _install_profhook():
    """Register the axon NTFF profile hook (missing glue in this container)."""
    import contextlib
    import ctypes
    import sys
    import types

    if "antenv.axon_hooks" in sys.modules:
        return
    try:
        lib = ctypes.CDLL("/opt/axon/libaxon_pjrt.so")
        assert hasattr(lib, "axon_start_nrt_profile")
    except Exception:
        return
    lib.axon_start_nrt_profile.argtypes = [ctypes.POINTER(ctypes.c_int64),
                                           ctypes.c_size_t]
    lib.axon_start_nrt_profile.restype = ctypes.c_int64
    lib.axon_stop_nrt_profile.argtypes = [ctypes.c_char_p]
    lib.axon_stop_nrt_profile.restype = ctypes.c_int64

    @contextlib.contextmanager
    def _hook(output_dir, device_ids):
        import jax

        jax.devices()
        if device_ids:
            ids = (ctypes.c_int64 * len(device_ids))(*device_ids)
            rc = lib.axon_start_nrt_profile(ids, len(device_ids))
        else:
            rc = lib.axon_start_nrt_profile(None, 0)
        if rc != 0:
            raise RuntimeError(f"axon_start_nrt_profile rc={rc}")
        try:
            yield
        finally:
            lib.axon_stop_nrt_profile(str(output_dir).encode())

    mod = types.ModuleType("antenv.axon_hooks")
    mod.get_axon_ntff_profile_hook = lambda: _hook
    mod.set_axon_ntff_profile_hook = lambda h: None
    sys.modules["antenv.axon_hooks"] = mod
    import antenv

    antenv.axon_hooks = mod


def kernel(x, edge_index, w_i, w_j):
    import os
    from concourse.bass_utils import run_bass_kernel_spmd

    x = np.asarray(x, dtype=np.float32)
    edge_index = np.asarray(edge_index)
    w_i = np.asarray(w_i, dtype=np.float32)
    w_j = np.asarray(w_j, dtype=np.float32)
    n = x.shape[0]
    assert n == N_NODES and x.shape[1] == HIDDEN
    npc = n // N_CORES

    ej = edge_index[0].astype(np.int64)
    ei = edge_index[1].astype(np.int64)
    core_of = ei // npc
    edge_src, edge_dstl = [], []
    for c in range(N_CORES):
        sel = core_of == c
        edge_src.append(ej[sel])
        edge_dstl.append(ei[sel] - c * npc)

    bucket_list, total_tiles, total_cols, Js, colmaps = _build_layout(
        edge_src, edge_dstl, npc)
    blockmasks, selmasks = _build_masks(bucket_list)

    xpad = np.zeros((NPAD, HIDDEN), dtype=np.float32)
    xpad[:n] = x
    XRfull = np.zeros((P, BCOLS * HIDDEN), dtype=np.float32)
    XRfull[:, :(BCOLS - 1) * HIDDEN] = np.ascontiguousarray(
        xpad.reshape(BCOLS - 1, P, HIDDEN).transpose(1, 0, 2)
    ).reshape(P, (BCOLS - 1) * HIDDEN)
    XT = np.zeros((HIDDEN, P * BCOLS), dtype=np.float32)
    XT[:, :NPAD] = xpad.T
    W2 = np.stack([w_j, w_i], axis=1).astype(np.float32)

    nc = _build_program(bucket_list, total_tiles, total_cols,
                        blockmasks.shape[1])
    in_maps = [{
        "XR": XRfull, "XT": XT, "W2": W2,
        "JT": Js[c],
        "BM": np.ascontiguousarray(blockmasks),
        "SM": np.ascontiguousarray(selmasks),
    } for c in range(N_CORES)]
    trace = os.environ.get("GAT_TRACE") == "1"
    if trace:
        _install_profhook()
    res = run_bass_kernel_spmd(nc, in_maps, core_ids=list(range(N_CORES)),
                               trace=trace)
    if trace and res.exec_time_ns:
        print(f"HW exec time: {res.exec_time_ns} ns")

    out = np.zeros((n, HIDDEN), dtype=np.float32)
    for c in range(N_CORES):
        ot = res.results[c]["OUT"][1:]
        cm = colmaps[c]
        valid = cm >= 0
        out[c * npc + cm[valid]] = ot[:, valid].T
    return out



# revision 9
# speedup vs baseline: 1.3537x; 1.3537x over previous
"""GAT message-passing kernel for 8 TRN2 NeuronCores (Bass/Tile).

Strategy (dst-sharded, no collectives):
  - Each core owns a contiguous range of destination nodes; the host routes
    each edge to the core owning its destination (edge_index[1]).
  - Device phase A: build a node table T[row n] = [x[n] bf16 x64 | 1.0 | s_j |
    s_i | 0] (68 bf16 = 136B rows) in HBM. s_j/s_i = x @ w_j / x @ w_i are
    computed on TensorE from a host-transposed copy of x.
  - The host groups each core's edges into per-destination blocks of
    S = deg+1 slots (slot 0 gathers the destination's own row, providing
    s_i), buckets nodes by exact degree, packs blocks into 128-slot tiles,
    and emits one int32 table-row index per slot.
  - Device phase B: [128,1] indirect-DMA gathers pull 128 slot rows each
    into SBUF; ScalarE computes exp(leakyrelu(s_i + s_j)); TensorE
    mask-matmuls reduce each tile to per-node numerators (sum ex*x) and
    denominators (sum ex); divide + relu; DMA out.
  - Host inverts the block permutation and assembles the full output.
"""
import numpy as np

N_NODES = 100000
HIDDEN = 64
N_CORES = 8
LEAKY = 0.01
P = 128
BCOLS = 784              # table columns per partition (783 data + 1 sentinel)
NPAD = P * (BCOLS - 1)   # 100224 padded node count
ROW = 68                 # bf16 elements per table row (136B)
SENTINEL = BCOLS - 1     # flat table row 783 (partition 0, col 783)
TCHUNK = 16              # table-build chunk (columns per iteration)
NB = 16                  # gather tiles per sub-batch
NQUEUES = 4              # SWDGE queues to rotate gathers over (ucode max 4)
NEG_BIG = -1.0e30


def _phi(n):
    """node id -> flat table row index (partition-major storage)."""
    return (n % P) * BCOLS + n // P


def _build_layout(edge_src, edge_dst_local, nodes_per_core):
    ncores = len(edge_src)
    blocks = {}
    for c in range(ncores):
        src, dstl = edge_src[c], edge_dst_local[c]
        order = np.argsort(dstl, kind="stable")
        src, dstl = src[order], dstl[order]
        deg = np.bincount(dstl, minlength=nodes_per_core)
        starts = np.concatenate([[0], np.cumsum(deg)])
        per = {}
        for n in np.nonzero(deg)[0]:
            d = int(deg[n])
            per.setdefault(d, []).append((int(n), src[starts[n]:starts[n + 1]]))
        blocks[c] = per

    all_d = sorted({d for c in range(ncores) for d in blocks[c].keys()})
    bucket_list = []
    for d in all_d:
        if d <= 0 or d > 126:
            raise ValueError(f"unsupported degree {d}")
        S = d + 1
        m = P // S
        maxb = max(len(blocks[c].get(d, [])) for c in range(ncores))
        n_tiles = (maxb + m - 1) // m
        bucket_list.append((d, n_tiles, m))

    total_tiles = sum(t for _, t, _ in bucket_list)
    total_cols = sum(t * m for _, t, m in bucket_list)
    Js, colmaps = [], []
    for c in range(ncores):
        J = np.full((total_tiles, P), SENTINEL, dtype=np.int32)
        colmap = np.full(total_cols, -1, dtype=np.int32)
        k0 = 0
        c0 = 0
        base_global = c * nodes_per_core
        for d, n_tiles, m in bucket_list:
            S = d + 1
            for bi, (n, srcs) in enumerate(blocks[c].get(d, [])):
                t, b = bi // m, bi % m
                J[k0 + t, b * S] = _phi(base_global + n)
                J[k0 + t, b * S + 1: b * S + 1 + d] = _phi(srcs)
                colmap[c0 + t * m + b] = n
            k0 += n_tiles
            c0 += n_tiles * m
        Js.append(np.ascontiguousarray(J.T))
        colmaps.append(colmap)
    return bucket_list, total_tiles, total_cols, Js, colmaps


def _build_masks(bucket_list):
    import ml_dtypes

    bm, sm = [], []
    for d, _, m in bucket_list:
        S = d + 1
        B = np.zeros((P, m), dtype=np.float32)
        SEL = np.zeros((P, P), dtype=np.float32)
        for p in range(m * S):
            if p % S != 0:
                B[p, p // S] = 1.0
            SEL[(p // S) * S, p] = 1.0
        bm.append(B)
        sm.append(SEL)
    return (np.concatenate(bm, 1).astype(ml_dtypes.bfloat16),
            np.concatenate(sm, 1).astype(ml_dtypes.bfloat16))


def _build_program(bucket_list, total_tiles, total_cols, n_bm_cols):
    import concourse.bass as bass
    import concourse.tile as tile
    from concourse import bacc, mybir
    from concourse.mybir import ActivationFunctionType as AFT

    nc = bacc.Bacc("TRN2", target_bir_lowering=False,
                   num_swdge_queues=NQUEUES,
                   dynamic_dma_scratch_size=65536)
    assert nc.num_swdge_queues == NQUEUES
    XR = nc.dram_tensor("XR", [P, BCOLS * HIDDEN], mybir.dt.bfloat16,
                        kind="ExternalInput")
    XT = nc.dram_tensor("XT", [HIDDEN, P * BCOLS], mybir.dt.bfloat16,
                        kind="ExternalInput")
    W2 = nc.dram_tensor("W2", [HIDDEN, 2], mybir.dt.float32,
                        kind="ExternalInput")
    JT = nc.dram_tensor("JT", [P, total_tiles], mybir.dt.int32,
                        kind="ExternalInput")
    BM = nc.dram_tensor("BM", [P, n_bm_cols], mybir.dt.bfloat16,
                        kind="ExternalInput")
    SM = nc.dram_tensor("SM", [P, P * len(bucket_list)], mybir.dt.bfloat16,
                        kind="ExternalInput")
    T = nc.dram_tensor("T", [P, BCOLS * ROW], mybir.dt.bfloat16)
    OUT = nc.dram_tensor("OUT", [HIDDEN + 1, total_cols], mybir.dt.float32,
                         kind="ExternalOutput")

    Trows = T[:].rearrange("p (b c) -> (p b) c", c=ROW)
    Tview = T[:].rearrange("p (b c) -> p b c", c=ROW)

    with tile.TileContext(nc) as tc:
        # ---------------- phase A: build node table ----------------
        with (
            tc.tile_pool(name="xa", bufs=2) as xa,
            tc.tile_pool(name="xt", bufs=2) as xtp,
            tc.tile_pool(name="stg", bufs=2) as stg,
            tc.tile_pool(name="wp", bufs=1) as wp,
            tc.tile_pool(name="psA", bufs=2, space="PSUM") as psA,
        ):
            w2f = wp.tile([HIDDEN, 2], mybir.dt.float32)
            nc.sync.dma_start(w2f[:], W2[:])
            w2b = wp.tile([HIDDEN, 2], mybir.dt.bfloat16)
            nc.vector.tensor_copy(w2b[:], w2f[:])

            for it in range(BCOLS // TCHUNK):
                b0 = it * TCHUNK
                xin = xa.tile([P, TCHUNK * HIDDEN], mybir.dt.bfloat16)
                nc.sync.dma_start(
                    xin[:], XR[:, b0 * HIDDEN:(b0 + TCHUNK) * HIDDEN])
                xtb = xtp.tile([HIDDEN, TCHUNK * P], mybir.dt.bfloat16)
                nc.sync.dma_start(xtb[:], XT[:, b0 * P:(b0 + TCHUNK) * P])
                ps = psA.tile([P, 2 * TCHUNK], mybir.dt.float32)
                for j in range(TCHUNK):
                    nc.tensor.matmul(
                        ps[:, 2 * j:2 * j + 2],
                        lhsT=xtb[:, j * P:(j + 1) * P],
                        rhs=w2b[:],
                        start=True, stop=True)
                st = stg.tile([P, TCHUNK, ROW], mybir.dt.bfloat16)
                nc.vector.memset(st[:, :, 0:1], 1.0)
                nc.vector.tensor_copy(
                    st[:, :, 1:HIDDEN + 1],
                    xin[:].rearrange("p (t h) -> p t h", h=HIDDEN))
                nc.vector.tensor_copy(
                    st[:, :, HIDDEN + 1:HIDDEN + 3],
                    ps[:].rearrange("p (t s) -> p t s", s=2))
                nc.vector.memset(st[:, :, HIDDEN + 3:ROW], 0.0)
                nc.sync.dma_start(
                    T[:, b0 * ROW:(b0 + TCHUNK) * ROW],
                    st[:].rearrange("p t c -> p (t c)"))

            sent = wp.tile([P, 1], mybir.dt.bfloat16)
            nc.vector.memset(sent[:], NEG_BIG)
            nc.sync.dma_start(
                Tview[:, SENTINEL, HIDDEN + 1:HIDDEN + 2], sent[:])

        # ---------------- phase B: gather + softmax + reduce --------
        with (
            tc.tile_pool(name="msk", bufs=1) as mskp,
            tc.tile_pool(name="jt", bufs=2) as jtp,
            tc.tile_pool(name="g", bufs=3) as gp,
            tc.tile_pool(name="sc", bufs=4) as scp,
            tc.tile_pool(name="fl", bufs=4) as flp,
            tc.tile_pool(name="psS", bufs=2, space="PSUM") as psS,
            tc.tile_pool(name="psU", bufs=2, space="PSUM") as psU,
        ):
            bmall = mskp.tile([P, n_bm_cols], mybir.dt.bfloat16)
            nc.sync.dma_start(bmall[:], BM[:])
            small = mskp.tile([P, P * len(bucket_list)], mybir.dt.bfloat16)
            nc.sync.dma_start(small[:], SM[:])
            ones1 = mskp.tile([1, HIDDEN + 1], mybir.dt.bfloat16)
            nc.vector.memset(ones1[:], 1.0)

            k0 = 0
            c0 = 0
            bm0 = 0
            for bi, (d, n_tiles, m) in enumerate(bucket_list):
                t = 0
                while t < n_tiles:
                    nb = min(NB, max(1, 512 // m), n_tiles - t)
                    jt = jtp.tile([P, NB], mybir.dt.int32, tag="jt")
                    nc.sync.dma_start(jt[:, :nb], JT[:, k0 + t:k0 + t + nb])
                    G = gp.tile([P, NB, ROW], mybir.dt.bfloat16, tag="G")
                    for k in range(nb):
                        gi = nc.gpsimd.indirect_dma_start(
                            out=G[:, k, :],
                            out_offset=None,
                            in_=Trows,
                            in_offset=bass.IndirectOffsetOnAxis(
                                ap=jt[:, k:k + 1], axis=0))
                        q = k % NQUEUES
                        if q:
                            gi.queue = f"qPoolDynamic{q}"
                    Vc = scp.tile([P, NB], mybir.dt.bfloat16, tag="Vc")
                    nc.vector.tensor_copy(Vc[:, :nb], G[:, :nb, HIDDEN + 2])
                    sib = psS.tile([P, NB], mybir.dt.float32, tag="sib")
                    nc.tensor.matmul(
                        sib[:, :nb],
                        lhsT=small[:, bi * P:(bi + 1) * P],
                        rhs=Vc[:, :nb],
                        start=True, stop=True)
                    eraw = scp.tile([P, NB], mybir.dt.float32, tag="eraw")
                    nc.vector.tensor_add(
                        eraw[:, :nb], sib[:, :nb], G[:, :nb, HIDDEN + 1])
                    esc = scp.tile([P, NB], mybir.dt.float32, tag="esc")
                    nc.vector.tensor_scalar_mul(esc[:, :nb], eraw[:, :nb],
                                                LEAKY)
                    elr = scp.tile([P, NB], mybir.dt.float32, tag="elr")
                    nc.vector.tensor_max(elr[:, :nb], eraw[:, :nb],
                                         esc[:, :nb])
                    ex = scp.tile([P, NB], mybir.dt.float32, tag="ex")
                    nc.scalar.activation(ex[:, :nb], elr[:, :nb], AFT.Exp)
                    exsel = scp.tile([P, NB, m], mybir.dt.bfloat16,
                                     tag="exsel")
                    nc.vector.tensor_mul(
                        exsel[:, :nb, :],
                        bmall[:, bm0:bm0 + m].unsqueeze(1).broadcast_to(
                            [P, nb, m]),
                        ex[:, :nb].unsqueeze(2).broadcast_to([P, nb, m]))
                    U = psU.tile([HIDDEN + 1, 512], mybir.dt.float32,
                                 tag="U")
                    for k in range(nb):
                        nc.tensor.matmul(
                            U[:, k * m:(k + 1) * m],
                            lhsT=G[:, k, 0:HIDDEN + 1],
                            rhs=exsel[:, k, :],
                            start=True, stop=True)
                    dsb = flp.tile([1, 512], mybir.dt.float32, tag="dsb")
                    nc.vector.tensor_scalar_max(
                        dsb[:, :nb * m], U[0:1, :nb * m], 1e-30)
                    recf = flp.tile([1, 512], mybir.dt.float32, tag="recf")
                    nc.vector.reciprocal(recf[:, :nb * m], dsb[:, :nb * m])
                    rec = flp.tile([1, 512], mybir.dt.bfloat16, tag="rec")
                    nc.vector.tensor_copy(rec[:, :nb * m], recf[:, :nb * m])
                    rb = psS.tile([HIDDEN + 1, 512], mybir.dt.float32,
                                  tag="rb")
                    nc.tensor.matmul(
                        rb[:, :nb * m], lhsT=ones1[:], rhs=rec[:, :nb * m],
                        start=True, stop=True)
                    rbs = flp.tile([HIDDEN + 1, 512], mybir.dt.float32,
                                   tag="rbs")
                    nc.scalar.copy(rbs[:, :nb * m], rb[:, :nb * m])
                    ot = flp.tile([HIDDEN + 1, 512], mybir.dt.float32,
                                  tag="ot")
                    nc.vector.tensor_mul(
                        ot[:, :nb * m], U[:, :nb * m], rbs[:, :nb * m])
                    otr = flp.tile([HIDDEN + 1, 512], mybir.dt.float32,
                                   tag="otr")
                    nc.vector.tensor_scalar_max(
                        otr[:, :nb * m], ot[:, :nb * m], 0.0)
                    nc.sync.dma_start(
                        OUT[:, c0:c0 + nb * m], otr[:, :nb * m])
                    c0 += nb * m
                    t += nb
                k0 += n_tiles
                bm0 += m
    nc.compile()
    return nc


def _install_profhook():
    """Register the axon NTFF profile hook (missing glue in this container)."""
    import contextlib
    import ctypes
    import sys
    import types

    if "antenv.axon_hooks" in sys.modules:
        return
    try:
        lib = ctypes.CDLL("/opt/axon/libaxon_pjrt.so")
        assert hasattr(lib, "axon_start_nrt_profile")
    except Exception:
        return
    lib.axon_start_nrt_profile.argtypes = [ctypes.POINTER(ctypes.c_int64),
                                           ctypes.c_size_t]
    lib.axon_start_nrt_profile.restype = ctypes.c_int64
    lib.axon_stop_nrt_profile.argtypes = [ctypes.c_char_p]
    lib.axon_stop_nrt_profile.restype = ctypes.c_int64

    @contextlib.contextmanager
    def _hook(output_dir, device_ids):
        import jax

        jax.devices()
        if device_ids:
            ids = (ctypes.c_int64 * len(device_ids))(*device_ids)
            rc = lib.axon_start_nrt_profile(ids, len(device_ids))
        else:
            rc = lib.axon_start_nrt_profile(None, 0)
        if rc != 0:
            raise RuntimeError(f"axon_start_nrt_profile rc={rc}")
        try:
            yield
        finally:
            lib.axon_stop_nrt_profile(str(output_dir).encode())

    mod = types.ModuleType("antenv.axon_hooks")
    mod.get_axon_ntff_profile_hook = lambda: _hook
    mod.set_axon_ntff_profile_hook = lambda h: None
    sys.modules["antenv.axon_hooks"] = mod
    import antenv

    antenv.axon_hooks = mod


def kernel(x, edge_index, w_i, w_j):
    import os
    from concourse.bass_utils import run_bass_kernel_spmd

    x = np.asarray(x, dtype=np.float32)
    edge_index = np.asarray(edge_index)
    w_i = np.asarray(w_i, dtype=np.float32)
    w_j = np.asarray(w_j, dtype=np.float32)
    n = x.shape[0]
    assert n == N_NODES and x.shape[1] == HIDDEN
    npc = n // N_CORES

    ej = edge_index[0].astype(np.int64)
    ei = edge_index[1].astype(np.int64)
    core_of = ei // npc
    edge_src, edge_dstl = [], []
    for c in range(N_CORES):
        sel = core_of == c
        edge_src.append(ej[sel])
        edge_dstl.append(ei[sel] - c * npc)

    bucket_list, total_tiles, total_cols, Js, colmaps = _build_layout(
        edge_src, edge_dstl, npc)
    blockmasks, selmasks = _build_masks(bucket_list)

    import ml_dtypes

    xpad = np.zeros((NPAD, HIDDEN), dtype=np.float32)
    xpad[:n] = x
    xpad16 = xpad.astype(ml_dtypes.bfloat16)
    XRfull = np.zeros((P, BCOLS * HIDDEN), dtype=ml_dtypes.bfloat16)
    XRfull[:, :(BCOLS - 1) * HIDDEN] = np.ascontiguousarray(
        xpad16.reshape(BCOLS - 1, P, HIDDEN).transpose(1, 0, 2)
    ).reshape(P, (BCOLS - 1) * HIDDEN)
    XT = np.zeros((HIDDEN, P * BCOLS), dtype=ml_dtypes.bfloat16)
    XT[:, :NPAD] = xpad16.T
    W2 = np.stack([w_j, w_i], axis=1).astype(np.float32)

    nc = _build_program(bucket_list, total_tiles, total_cols,
                        blockmasks.shape[1])
    in_maps = [{
        "XR": XRfull, "XT": XT, "W2": W2,
        "JT": Js[c],
        "BM": np.ascontiguousarray(blockmasks),
        "SM": np.ascontiguousarray(selmasks),
    } for c in range(N_CORES)]
    trace = os.environ.get("GAT_TRACE") == "1"
    if trace:
        _install_profhook()
    res = run_bass_kernel_spmd(nc, in_maps, core_ids=list(range(N_CORES)),
                               trace=trace)
    if trace and res.exec_time_ns:
        print(f"HW exec time: {res.exec_time_ns} ns")

    out = np.zeros((n, HIDDEN), dtype=np.float32)
    for c in range(N_CORES):
        ot = res.results[c]["OUT"][1:]
        cm = colmaps[c]
        valid = cm >= 0
        out[c * npc + cm[valid]] = ot[:, valid].T
    return out

